# revision 4
# baseline (speedup 1.0000x reference)
"""Multi-head attention (B=4, T=2048, D=1024, H=16) on 8 trn2 NeuronCores.

Sharding: tensor-parallel over heads. Core c owns heads {2c, 2c+1}:
  - computes Q^T/K^T/V^T for its heads for ALL tokens (full x streamed,
    contraction over D done on the PE from host-pretransposed x^T),
  - attention in S^T layout ([k_partition, q_free]) so softmax needs no
    transposes: rowsum comes free as a ones-column appended to V in the
    P^T@V matmul; exp runs on the scalar engine straight out of PSUM,
  - AllToAll redistributes attn_out^T from head-sharding to token-sharding,
  - output projection is data-parallel over tokens (1/8 of tokens per core).

All matmuls run as float32r (FP22 mantissa, fp32 accumulate): 1 cycle/row
at free-dim 512, ~4x faster than true fp32.
"""

import sys

import numpy as np

for _p in ("/opt/trn_rl_repo",):
    if _p not in sys.path:
        sys.path.insert(0, _p)

import concourse.bass as bass  # noqa: E402
import concourse.tile as tile  # noqa: E402
from concourse import bacc, mybir  # noqa: E402
from concourse.bass_utils import run_bass_kernel_spmd  # noqa: E402
from concourse.masks import make_identity  # noqa: E402

FP = mybir.dt.float32
FPR = mybir.dt.float32r
P = 128
D = 1024
H = 16
DH = 64
NCORES = 8
HL = H // NCORES  # local heads per core
DL = HL * DH      # local head dims per core (128)
QB = 512          # q-block / token-block width
NDC = D // P      # contraction chunks over D


def _r(ap):
    return ap.bitcast(FPR)


def build_program(B, T):
    """Builds + compiles the SPMD program. Same program on all 8 cores."""
    NTOK = B * T
    TOKG = NTOK // NCORES  # tokens per core for the output projection
    NTB = T // QB          # token blocks per batch
    NKC = T // P           # k-chunks per batch
    NQB = T // QB          # q-blocks per batch
    AluOp = mybir.AluOpType
    Act = mybir.ActivationFunctionType

    nc = bacc.Bacc(
        "TRN2",
        target_bir_lowering=False,
        debug=False,
        num_devices=NCORES,
    )
    xT = nc.dram_tensor("xT", [D, NTOK], FP, kind="ExternalInput").ap()
    wT = nc.dram_tensor("wT", [D, 3 * DL], FP, kind="ExternalInput").ap()
    bqkv = nc.dram_tensor("bqkv", [3 * DL, 1], FP, kind="ExternalInput").ap()
    owT = nc.dram_tensor("owT", [D, D], FP, kind="ExternalInput").ap()
    ob = nc.dram_tensor("ob", [1, D], FP, kind="ExternalInput").ap()
    out = nc.dram_tensor("out", [TOKG, D], FP, kind="ExternalOutput").ap()

    with tile.TileContext(nc) as tc:
        with (
            tc.tile_pool(name="consts", bufs=1) as consts,
            tc.tile_pool(name="big", bufs=2) as big,
            tc.tile_pool(name="xtp", bufs=2) as xtp,
            tc.tile_pool(name="ptp", bufs=3) as ptp,
            tc.tile_pool(name="outp", bufs=3) as outp,
            tc.tile_pool(name="smallp", bufs=3) as smallp,
            tc.tile_pool(name="a2ap", bufs=8) as a2ap,
            tc.tile_pool(name="finp", bufs=3) as finp,
            tc.tile_pool(name="psum_a", bufs=3, space="PSUM") as psum_a,
            tc.tile_pool(name="psum_s", bufs=3, space="PSUM") as psum_s,
            tc.tile_pool(name="psum_pv", bufs=2, space="PSUM") as psum_pv,
            tc.tile_pool(name="dram", bufs=1, space="DRAM") as dram,
        ):
            # ---- constants / weights resident in SBUF
            ident = consts.tile([P, P], FP)
            make_identity(nc, ident)
            ones64 = consts.tile([1, DH], FPR)
            nc.scalar.activation(
                ones64, ident[0:1, 0:DH], Act.Copy, bias=1.0, scale=0.0
            )

            wT_sb = consts.tile([P, NDC, 3 * DL], FPR)
            for dc in range(NDC):
                nc.sync.dma_start(
                    out=wT_sb[:, dc, :], in_=wT[dc * P:(dc + 1) * P, :].bitcast(FPR)
                )
            bias_sb = consts.tile([P, 3], FP)
            for i in range(3):
                nc.sync.dma_start(out=bias_sb[:, i:i + 1], in_=bqkv[i * P:(i + 1) * P, :])
            owT_sb = consts.tile([P, NDC, D], FPR)
            for dc in range(NDC):
                nc.sync.dma_start(
                    out=owT_sb[:, dc, :], in_=owT[dc * P:(dc + 1) * P, :].bitcast(FPR)
                )
            bias_bc = consts.tile([P, D], FP)
            nc.gpsimd.dma_start(
                out=bias_bc,
                in_=bass.AP(tensor=ob.tensor, offset=ob.offset, ap=[[0, P], [1, D]]),
            )

            a2a_in = dram.tile([NCORES, DL, TOKG], FP)
            a2a_out = dram.tile([NCORES, DL, TOKG], FP)

            for b in range(B):
                # ---- QKV^T projection for batch b  (out: [128 rows, T])
                kT = big.tile([P, T], FPR, tag="kT")
                vT = big.tile([P, T], FP, tag="vT")
                qT = big.tile([P, T], FPR, tag="qT")
                vt = big.tile([P, NKC, 2 * (DH + 1)], FPR, tag="vt")
                nc.scalar.activation(
                    vt[:, :, DH:DH + 1], vt[:, :, DH:DH + 1].bitcast(FP),
                    Act.Copy, bias=1.0, scale=0.0,
                )
                nc.scalar.activation(
                    vt[:, :, 2 * DH + 1:2 * DH + 2],
                    vt[:, :, 2 * DH + 1:2 * DH + 2].bitcast(FP),
                    Act.Copy, bias=1.0, scale=0.0,
                )
                for tb in range(NTB):
                    tok0 = b * T + tb * QB
                    xt = xtp.tile([P, NDC, QB], FPR, tag="xt")
                    for dc in range(NDC):
                        nc.sync.dma_start(
                            out=xt[:, dc, :],
                            in_=xT[dc * P:(dc + 1) * P, tok0:tok0 + QB].bitcast(FPR),
                        )
                    for i, dst in enumerate((qT, kT, vT)):
                        ps = psum_a.tile([P, QB], FP, tag="mm")
                        for dc in range(NDC):
                            nc.tensor.matmul(
                                ps,
                                wT_sb[:, dc, i * DL:(i + 1) * DL],
                                xt[:, dc, :],
                                start=(dc == 0),
                                stop=(dc == NDC - 1),
                            )
                        # q is pre-scaled by 1/sqrt(dh); host passes bias_q/8.
                        # ACT rounds the output to FP32R for the PE.
                        if i < 2:
                            nc.scalar.activation(
                                dst[:, tb * QB:(tb + 1) * QB],
                                ps,
                                Act.Identity,
                                bias=bias_sb[:, i:i + 1],
                                scale=0.125 if i == 0 else 1.0,
                            )
                        else:
                            nc.vector.tensor_scalar(
                                out=dst[:, tb * QB:(tb + 1) * QB],
                                in0=ps,
                                scalar1=1.0,
                                scalar2=bias_sb[:, i:i + 1],
                                op0=AluOp.mult,
                                op1=AluOp.add,
                            )
                    # transpose this block of V^T into [token, d] tiles (+ones col)
                    for j in range(QB // P):
                        kc = tb * (QB // P) + j
                        pst = psum_a.tile([P, P], FP, tag="mm")
                        nc.tensor.transpose(
                            pst, vT[:, tb * QB + j * P:tb * QB + (j + 1) * P], ident
                        )
                        nc.scalar.activation(vt[:, kc, 0:DH], pst[:, 0:DH], Act.Copy)
                        nc.scalar.activation(
                            vt[:, kc, DH + 1:2 * DH + 1], pst[:, DH:2 * DH], Act.Copy
                        )

                # ---- attention for batch b
                for qb in range(NQB):
                    q0 = qb * QB
                    for h in range(HL):
                        pv = psum_pv.tile([P, QB], FP, tag="pv")
                        for kc in range(NKC):
                            ss = psum_s.tile([P, QB], FP, tag="s")
                            nc.tensor.matmul(
                                ss,
                                kT[h * DH:(h + 1) * DH, kc * P:(kc + 1) * P],
                                qT[h * DH:(h + 1) * DH, q0:q0 + QB],
                                start=True,
                                stop=True,
                            )
                            pt = ptp.tile([P, QB], FPR, tag="pt")
                            nc.scalar.activation(pt, ss, Act.Exp)
                            nc.tensor.matmul(
                                pv[:DH + 1, :],
                                vt[:, kc, h * (DH + 1):(h + 1) * (DH + 1)],
                                pt,
                                start=(kc == 0),
                                stop=(kc == NKC - 1),
                            )
                        # normalize by the softmax denominator (row DH of pv)
                        rec32 = smallp.tile([1, QB], FP, tag="rec32")
                        nc.vector.reciprocal(rec32, pv[DH:DH + 1, :])
                        rec = smallp.tile([1, QB], FPR, tag="rec")
                        nc.scalar.activation(rec, rec32, Act.Copy)
                        bc = psum_s.tile([DH, QB], FP, tag="s")
                        nc.tensor.matmul(bc, ones64, rec, start=True, stop=True)
                        bc_sb = outp.tile([DH, QB], FP, tag="bcs")
                        nc.vector.tensor_copy(bc_sb, bc)
                        outn = outp.tile([DH, QB], FP, tag="outn")
                        nc.vector.tensor_mul(outn, pv[0:DH, :], bc_sb)
                        flat0 = b * T + q0
                        step = min(QB, TOKG)
                        for o in range(0, QB, step):
                            g = (flat0 + o) // TOKG
                            col = (flat0 + o) % TOKG
                            nc.sync.dma_start(
                                out=a2a_in[g, h * DH:(h + 1) * DH, col:col + step],
                                in_=outn[:, o:o + step],
                            )

            # ---- redistribute: head-sharded attn_out^T -> token-sharded
            nc.gpsimd.collective_compute(
                "AllToAll",
                AluOp.bypass,
                replica_groups=[list(range(NCORES))],
                ins=[a2a_in.opt()],
                outs=[a2a_out.opt()],
            )

            # ---- output projection for this core's TOKG tokens
            for t8 in range(TOKG // P):
                ats = []
                for s in range(NCORES):
                    at = a2ap.tile([P, P], FPR, tag="at")
                    nc.sync.dma_start(
                        out=at, in_=a2a_out[s, :, t8 * P:(t8 + 1) * P].bitcast(FPR)
                    )
                    ats.append(at)
                for nb in range(D // QB):
                    ps = psum_a.tile([P, QB], FP, tag="mm")
                    for s in range(NCORES):
                        nc.tensor.matmul(
                            ps,
                            ats[s],
                            owT_sb[:, s, nb * QB:(nb + 1) * QB],
                            start=(s == 0),
                            stop=(s == NCORES - 1),
                        )
                    fin = finp.tile([P, QB], FP, tag="fin")
                    nc.vector.tensor_add(fin, ps, bias_bc[:, nb * QB:(nb + 1) * QB])
                    nc.sync.dma_start(
                        out=out[t8 * P:(t8 + 1) * P, nb * QB:(nb + 1) * QB], in_=fin
                    )

    nc.compile()
    return nc


def make_in_maps(x, qkv_w, qkv_b, out_w, out_b):
    """Host-side sharding: pre-transposed activations/weights per core."""
    B, T, _ = x.shape
    xT_np = np.ascontiguousarray(x.reshape(B * T, D).T).astype(np.float32)
    owT_np = np.ascontiguousarray(out_w.T).astype(np.float32)
    ob_np = np.ascontiguousarray(out_b.reshape(1, D)).astype(np.float32)
    in_maps = []
    for c in range(NCORES):
        r0, r1 = 2 * c * DH, (2 * c + HL) * DH  # local head rows (128 wide)
        w_rows = np.concatenate(
            [qkv_w[r0:r1], qkv_w[D + r0:D + r1], qkv_w[2 * D + r0:2 * D + r1]], axis=0
        )
        wT_c = np.ascontiguousarray(w_rows.T).astype(np.float32)
        b_c = np.concatenate(
            [qkv_b[r0:r1] / 8.0, qkv_b[D + r0:D + r1], qkv_b[2 * D + r0:2 * D + r1]]
        ).reshape(3 * DL, 1).astype(np.float32)
        in_maps.append(
            {"xT": xT_np, "wT": wT_c, "bqkv": b_c, "owT": owT_np, "ob": ob_np}
        )
    return in_maps


_PROGRAM_CACHE = {}


def _get_program(B, T):
    key = (B, T)
    if key not in _PROGRAM_CACHE:
        _PROGRAM_CACHE[key] = build_program(B, T)
    return _PROGRAM_CACHE[key]


def run_on_hw(x, qkv_w, qkv_b, out_w, out_b, trace=False):
    B, T, _ = x.shape
    nc = _get_program(B, T)
    in_maps = make_in_maps(x, qkv_w, qkv_b, out_w, out_b)
    res = run_bass_kernel_spmd(nc, in_maps, core_ids=list(range(NCORES)), trace=trace)
    outs = [res.results[c]["out"] for c in range(NCORES)]
    full = np.concatenate(outs, axis=0).reshape(B, T, D).astype(np.float32)
    return full, res


def kernel(x, qkv_w, qkv_b, out_w, out_b):
    x = np.asarray(x, dtype=np.float32)
    qkv_w = np.asarray(qkv_w, dtype=np.float32)
    qkv_b = np.asarray(qkv_b, dtype=np.float32)
    out_w = np.asarray(out_w, dtype=np.float32)
    out_b = np.asarray(out_b, dtype=np.float32)
    full, _ = run_on_hw(x, qkv_w, qkv_b, out_w, out_b)
    return full
